# revision 19
# baseline (speedup 1.0000x reference)
"""Multi-head GNN attention message-passing kernel for 8 TRN2 NeuronCores.

Strategy (edge-parallel, dst-sorted):
  - Sort edges by dst on host; split dst-node space into 8 contiguous,
    128-aligned per-core ranges (edge counts balance to ~1% for uniform dst).
  - Phase 1 (per core, replicated): K|V projections for ALL nodes written to
    per-core HBM tables in bf16 (lo/hi split keeps gather indices in int16);
    Q projection for the core's own dst range stays resident in SBUF.
  - Phase 2 (per core): per 128-dst-node window, bulk-gather K|V rows of the
    window's edges (dma_gather, <=512-index single-packet chunks).  Per-edge
    Q rows are NOT gathered: S1[e,n]=(dst_local[e]==n) is built on DVE, PE
    transposes it to S2 and computes Q_edges = S2 @ Q_win; ACT moves PSUM
    results back to SBUF.  Scores exp(clip(K.Q/4)) on DVE/ACT, segment-sum of
    messages+scores via PE matmuls (lhsT=S1) accumulating in PSUM, epilogue
    divides by z.  No collectives: every core owns its dst range outright.
"""

import math
from dataclasses import dataclass

import numpy as np

P = 128
H = 8
D = 16
HD = H * D  # 128
IN_DIM = 128
LO_CAP = 32768  # rows per gather table must stay below int16 positive range


@dataclass(frozen=True)
class Cfg:
    n: int        # true node count
    ncores: int
    nw: int       # windows (128 dst nodes each) per core
    s_lo: int     # subtiles (128 edges) per window gathered from the lo table
    s_hi: int     # subtiles per window gathered from the hi table
    lo_n: int     # node rows in the lo KV table (window aligned)
    gchunk: int = 6   # subtiles per dma_gather chunk (768 idx per packet)
    sgrp: int = 4     # subtiles per Q_edges PSUM group (4*128 f32 = one bank)
    nq: int = 4       # SWDGE queues; chunks alternate queues in issue order

    @property
    def nloc(self) -> int:
        return self.nw * P

    @property
    def np_(self) -> int:
        return self.nloc * self.ncores

    @property
    def nwg(self) -> int:
        return self.np_ // P

    @property
    def sw(self) -> int:
        return self.s_lo + self.s_hi

    @property
    def hi_n(self) -> int:
        return self.np_ - self.lo_n


def _wrap_idx(idx: np.ndarray) -> np.ndarray:
    """[num] -> [128, num//16] int16 in the dma_gather wrapped+replicated layout."""
    w = idx.astype(np.int16).reshape(-1, 16).T  # [16, num//16]
    return np.tile(w, (8, 1))                   # [128, num//16]


def _bf16(a):
    import ml_dtypes

    return np.asarray(a, dtype=np.float32).astype(ml_dtypes.bfloat16)


def preprocess(h, Wq, bq, Wk, bk, Wv, bv, src, dst, ncores=8):
    """Host-side sharding. Returns (cfg, shared_inputs, per_core_inputs)."""
    n = h.shape[0]
    e = src.shape[0]
    nloc = int(math.ceil(n / (ncores * P))) * P
    np_ = nloc * ncores
    nw = nloc // P
    nwg = np_ // P
    lo_n = min(LO_CAP, np_)

    order = np.argsort(dst, kind="stable")
    dsts = np.asarray(dst)[order].astype(np.int64)
    srcs = np.asarray(src)[order].astype(np.int64)

    wb = np.searchsorted(dsts, np.arange(nwg + 1) * P)
    is_hi = srcs >= lo_n
    cnt = wb[1:] - wb[:-1]
    cnt_hi = np.add.reduceat(
        np.concatenate([is_hi.astype(np.int64), [0]]), np.minimum(wb[:-1], e)
    )
    cnt_hi = np.where(cnt > 0, cnt_hi, 0)[:nwg]
    cnt_lo = cnt - cnt_hi
    s_lo = max(1, int(math.ceil(cnt_lo.max() / P)))
    s_hi = int(math.ceil(cnt_hi.max() / P)) if lo_n < np_ else 0
    cfg = Cfg(n=n, ncores=ncores, nw=nw, s_lo=s_lo, s_hi=s_hi, lo_n=lo_n)
    sw = cfg.sw

    f32 = np.float32
    hT = np.zeros((IN_DIM, np_), dtype=f32)
    hT[:, :n] = np.asarray(h, dtype=f32).T
    hTb = _bf16(hT)
    ident = np.eye(P, dtype=f32)
    shared = {
        "hT": hTb,
        "Wkv": _bf16(np.hstack([np.asarray(Wk, f32), np.asarray(Wv, f32)])),
        "Wq_": _bf16(Wq),
        "bkv": _bf16(np.concatenate([np.asarray(bk, f32), np.asarray(bv, f32)])[None, :]),
        "bq_": _bf16(np.asarray(bq, f32)[None, :]),
        "ones1": _bf16(np.ones((1, P), f32)),
        "iota": _bf16(np.tile(np.arange(P, dtype=f32), (P, 1))),
        "ident": _bf16(ident),
    }

    per_core = []
    for c in range(ncores):
        kvlo = np.zeros((nw, P, s_lo * 8), np.int16)
        kvhi = np.zeros((nw, P, max(s_hi, 1) * 8), np.int16)
        dloc = np.full((nw, sw * P), 200.0, f32)
        for w in range(nw):
            g = c * nw + w
            lo_e, hi_e = wb[g], wb[g + 1]
            seg_s = srcs[lo_e:hi_e]
            seg_d = dsts[lo_e:hi_e] - g * P
            m_hi = seg_s >= lo_n
            s_l, d_l = seg_s[~m_hi], seg_d[~m_hi]
            s_h, d_h = seg_s[m_hi] - lo_n, seg_d[m_hi]
            il = np.zeros(s_lo * P, np.int64)
            il[: len(s_l)] = s_l
            kvlo[w] = _wrap_idx(il)
            if s_hi:
                ih = np.zeros(s_hi * P, np.int64)
                ih[: len(s_h)] = s_h
                kvhi[w] = _wrap_idx(ih)
            dloc[w, : len(d_l)] = d_l
            dloc[w, s_lo * P : s_lo * P + len(d_h)] = d_h
        per_core.append(
            {
                "kvloidx": kvlo,
                "kvhiidx": kvhi,
                # [nw, sw*P] slot-major -> [nw, P, sw] partition-major
                "dstloc": _bf16(dloc.reshape(nw, sw, P).transpose(0, 2, 1)),
                "hTloc": np.ascontiguousarray(hTb[:, c * nloc : (c + 1) * nloc]),
            }
        )
    return cfg, shared, per_core


def build_program(cfg: Cfg):
    """Builds the SPMD Bacc program for one core (same program on all cores)."""
    import concourse.bacc as bacc
    import concourse.mybir as mybir
    import concourse.tile as tile

    F32 = mybir.dt.float32
    BF16 = mybir.dt.bfloat16
    I16 = mybir.dt.int16
    AO = mybir.AluOpType
    AF = mybir.ActivationFunctionType

    nc = bacc.Bacc(
        "TRN2",
        target_bir_lowering=False,
        debug=False,
        num_devices=cfg.ncores,
        num_swdge_queues=cfg.nq,
    )

    np_, nloc, nw, nwg = cfg.np_, cfg.nloc, cfg.nw, cfg.nwg
    s_lo, s_hi, sw = cfg.s_lo, cfg.s_hi, cfg.sw
    lo_nw = cfg.lo_n // P  # windows that go to the lo table

    # ---- kernel I/O ----
    hT_d = nc.dram_tensor("hT", [IN_DIM, np_], BF16, kind="ExternalInput")
    hTloc_d = nc.dram_tensor("hTloc", [IN_DIM, nloc], BF16, kind="ExternalInput")
    Wkv_d = nc.dram_tensor("Wkv", [IN_DIM, 2 * HD], BF16, kind="ExternalInput")
    Wq_d = nc.dram_tensor("Wq_", [IN_DIM, HD], BF16, kind="ExternalInput")
    bkv_d = nc.dram_tensor("bkv", [1, 2 * HD], BF16, kind="ExternalInput")
    bq_d = nc.dram_tensor("bq_", [1, HD], BF16, kind="ExternalInput")
    ones_d = nc.dram_tensor("ones1", [1, P], BF16, kind="ExternalInput")
    iota_d = nc.dram_tensor("iota", [P, P], BF16, kind="ExternalInput")
    ident_d = nc.dram_tensor("ident", [P, P], BF16, kind="ExternalInput")
    kvlo_i_d = nc.dram_tensor("kvloidx", [nw, P, s_lo * 8], I16, kind="ExternalInput")
    kvhi_i_d = nc.dram_tensor(
        "kvhiidx", [nw, P, max(s_hi, 1) * 8], I16, kind="ExternalInput"
    )
    dstloc_d = nc.dram_tensor("dstloc", [nw, P, sw], BF16, kind="ExternalInput")
    out_d = nc.dram_tensor("out", [nloc, HD], F32, kind="ExternalOutput")

    # ---- internal HBM scratch ----
    KVlo_d = nc.dram_tensor("KVlo", [cfg.lo_n, 2 * HD], BF16, kind="Internal")
    if s_hi:
        KVhi_d = nc.dram_tensor("KVhi", [cfg.hi_n, 2 * HD], BF16, kind="Internal")

    _swdge_ctr = [0]
    _kv_fence = [None]

    def chunked_gather(table_d, idx_t, kv3, sub_off, nsub, elem):
        """Gather nsub*128 rows in <=gchunk-subtile single-packet chunks,
        alternating SWDGE queues in strict issue order (matches Tile's
        DMASW sem-lane rotation)."""
        off = 0
        while off < nsub:
            gc = min(cfg.gchunk, nsub - off)
            ga = nc.gpsimd.dma_gather(
                out_ap=kv3[:, sub_off + off : sub_off + off + gc, :],
                in_ap=table_d[:, :],
                idxs_ap=idx_t[:, off * 8 : (off + gc) * 8],
                num_idxs=gc * P,
                num_idxs_reg=gc * P,
                elem_size=elem,
                single_packet=True,
                queue_num=_swdge_ctr[0] % cfg.nq,
            )
            if _kv_fence[0] is not None:
                tile.add_dep_helper(ga.ins, _kv_fence[0].ins, reason="gather>kv")
            _swdge_ctr[0] += 1
            off += gc

    kv_writes = []

    with tile.TileContext(nc) as tc:
        with (
            tc.tile_pool(name="consts", bufs=1) as p_c,
            tc.tile_pool(name="p1", bufs=4) as p_1,
            tc.tile_pool(name="gath", bufs=3) as p_g,
            tc.tile_pool(name="work", bufs=3) as p_wk,
            tc.tile_pool(name="s2s", bufs=3) as p_s2,
            tc.tile_pool(name="epi", bufs=2) as p_epi,
        ):
            # constants
            wkv_t = p_c.tile([P, 2 * HD], BF16)
            nc.sync.dma_start(out=wkv_t[:], in_=Wkv_d[:, :])
            wq_t = p_c.tile([P, HD], BF16)
            nc.sync.dma_start(out=wq_t[:], in_=Wq_d[:, :])
            bkv_t = p_c.tile([1, 2 * HD], BF16)
            nc.sync.dma_start(out=bkv_t[:], in_=bkv_d[:, :])
            bq_t = p_c.tile([1, HD], BF16)
            nc.sync.dma_start(out=bq_t[:], in_=bq_d[:, :])
            ones_t = p_c.tile([1, P], BF16)
            nc.sync.dma_start(out=ones_t[:], in_=ones_d[:, :])
            iota_t = p_c.tile([P, P], BF16)
            nc.sync.dma_start(out=iota_t[:], in_=iota_d[:, :])
            ident_t = p_c.tile([P, P], BF16)
            nc.sync.dma_start(out=ident_t[:], in_=ident_d[:, :])
            # Q for the whole local dst range stays in SBUF (nw*256B/partition)
            q_all = p_c.tile([P, nw * HD], BF16)
            # bias replicated across partitions once; folded into psum->sbuf adds
            bkv_rep = p_c.tile([P, 2 * HD], BF16)
            nc.gpsimd.partition_broadcast(bkv_rep[:], bkv_t[:1, :])
            bq_rep = p_c.tile([P, HD], BF16)
            nc.gpsimd.partition_broadcast(bq_rep[:], bq_t[:1, :])

            # ---- phase 1: K|V for all nodes (4 windows per hT DMA) ----
            p_1ps_cm = tc.tile_pool(name="p1ps", bufs=2, space="PSUM")
            p_1ps = p_1ps_cm.__enter__()
            assert lo_nw % 4 == 0
            for g4 in range(0, nwg, 4):
                gn = min(4, nwg - g4)
                ht4 = p_1.tile([P, 4 * P], BF16, tag="ht")
                nc.sync.dma_start(
                    out=ht4[:, : gn * P], in_=hT_d[:, g4 * P : (g4 + gn) * P]
                )
                kv_sb4 = p_1.tile([P, 4 * 2 * HD], BF16, tag="kvsb")
                for j in range(gn):
                    ps = p_1ps.tile([P, 2 * HD], F32, tag="p1ps")
                    nc.tensor.matmul(
                        out=ps[:], lhsT=ht4[:, j * P : (j + 1) * P], rhs=wkv_t[:],
                        start=True, stop=True,
                    )
                    nc.vector.tensor_tensor(
                        out=kv_sb4[:, j * 2 * HD : (j + 1) * 2 * HD],
                        in0=ps[:], in1=bkv_rep[:], op=AO.add,
                    )
                # one batched 4-window store, viewed [P, gn, 256] -> rows
                kv4v = kv_sb4[:].rearrange("p (j e) -> p j e", e=2 * HD)[:, :gn, :]
                if g4 + gn <= lo_nw:
                    wr = nc.sync.dma_start(
                        out=KVlo_d[g4 * P : (g4 + gn) * P, :].rearrange(
                            "(j p) e -> p j e", p=P
                        ),
                        in_=kv4v,
                    )
                else:
                    gg = g4 - lo_nw
                    wr = nc.sync.dma_start(
                        out=KVhi_d[gg * P : (gg + gn) * P, :].rearrange(
                            "(j p) e -> p j e", p=P
                        ),
                        in_=kv4v,
                    )
                kv_writes.append(wr)

            # ---- phase 1b: Q for the local dst range -> resident SBUF ----
            for w4 in range(0, nw, 4):
                wn = min(4, nw - w4)
                ht4 = p_1.tile([P, 4 * P], BF16, tag="ht")
                nc.sync.dma_start(
                    out=ht4[:, : wn * P], in_=hTloc_d[:, w4 * P : (w4 + wn) * P]
                )
                for j in range(wn):
                    w = w4 + j
                    psq_full = p_1ps.tile([P, 2 * HD], F32, tag="p1ps")
                    psq = psq_full[:, :HD]
                    nc.tensor.matmul(
                        out=psq[:], lhsT=ht4[:, j * P : (j + 1) * P], rhs=wq_t[:],
                        start=True, stop=True,
                    )
                    nc.vector.tensor_tensor(
                        out=q_all[:, w * HD : (w + 1) * HD],
                        in0=psq[:], in1=bq_rep[:], op=AO.add,
                    )

            p_1ps_cm.__exit__(None, None, None)
            # Tile does not track RAW deps through DRAM: every gather must
            # follow every KV-table write.  A single fence NOP collapses the
            # 441x392 edge product; all other engines flow freely across it.
            kv_fence = nc.sync.nop()
            for wr in kv_writes:
                tile.add_dep_helper(kv_fence.ins, wr.ins, reason="kv fence")
            _kv_fence[0] = kv_fence

            p_s2ps_cm = tc.tile_pool(name="s2ps", bufs=2, space="PSUM")
            p_s2ps = p_s2ps_cm.__enter__()
            p_qeps_cm = tc.tile_pool(name="qeps", bufs=3, space="PSUM")
            p_qeps = p_qeps_cm.__enter__()
            p_2ps_cm = tc.tile_pool(name="p2ps", bufs=3, space="PSUM")
            p_2ps = p_2ps_cm.__enter__()

            # ---- phase 2: per-window edge processing ----
            for w in range(nw):
                il_t = p_g.tile([P, s_lo * 8], I16, tag="il")
                nc.sync.dma_start(out=il_t[:], in_=kvlo_i_d[w])
                if s_hi:
                    ih_t = p_g.tile([P, s_hi * 8], I16, tag="ih")
                    nc.sync.dma_start(out=ih_t[:], in_=kvhi_i_d[w])
                dl_t = p_g.tile([P, sw], BF16, tag="dl")
                nc.sync.dma_start(out=dl_t[:], in_=dstloc_d[w])

                kv_t = p_g.tile([P, sw * 2 * HD], BF16, tag="kv")
                kv3 = kv_t[:].rearrange("p (s e) -> p s e", e=2 * HD)
                chunked_gather(KVlo_d, il_t, kv3, 0, s_lo, 2 * HD)
                if s_hi:
                    chunked_gather(KVhi_d, ih_t, kv3, s_lo, s_hi, 2 * HD)

                # one-hot scatter indicator S1[e, n]
                s1 = p_wk.tile([P, sw * P], BF16, tag="s1")
                s13 = s1[:].rearrange("p (s n) -> p s n", n=P)
                nc.vector.tensor_tensor(
                    out=s13,
                    in0=dl_t[:].unsqueeze(2).to_broadcast([P, sw, P]),
                    in1=iota_t[:].unsqueeze(1).to_broadcast([P, sw, P]),
                    op=AO.is_equal,
                )
                # Q_edges = S1^T @ Q_win via PE, in groups of sgrp subtiles
                qw = q_all[:, w * HD : (w + 1) * HD]
                qe = p_wk.tile([P, sw * HD], BF16, tag="qe")
                for g0 in range(0, sw, cfg.sgrp):
                    g1 = min(g0 + cfg.sgrp, sw)
                    s2ps4 = p_s2ps.tile([P, cfg.sgrp * P], BF16, tag="s2ps")
                    for s in range(g0, g1):
                        nc.tensor.transpose(
                            out=s2ps4[:, (s - g0) * P : (s - g0 + 1) * P],
                            in_=s13[:, s, :],
                            identity=ident_t[:],
                        )
                    s2sb4 = p_s2.tile([P, cfg.sgrp * P], BF16, tag="s2sb")
                    nc.scalar.activation(
                        out=s2sb4[:, : (g1 - g0) * P],
                        in_=s2ps4[:, : (g1 - g0) * P],
                        func=AF.Copy,
                    )
                    qeps = p_qeps.tile([P, cfg.sgrp * HD], F32, tag="qeps")
                    for s in range(g0, g1):
                        nc.tensor.matmul(
                            out=qeps[:, (s - g0) * HD : (s - g0 + 1) * HD],
                            lhsT=s2sb4[:, (s - g0) * P : (s - g0 + 1) * P],
                            rhs=qw,
                            start=True,
                            stop=True,
                        )
                    nc.scalar.activation(
                        out=qe[:, g0 * HD : g1 * HD],
                        in_=qeps[:, : (g1 - g0) * HD],
                        func=AF.Copy,
                    )

                # scores
                kq = p_wk.tile([P, sw * HD], BF16, tag="kq")
                kq3 = kq[:].rearrange("p (s e) -> p s e", e=HD)
                nc.vector.tensor_tensor(
                    out=kq3,
                    in0=kv3[:, :, 0:HD],
                    in1=qe[:].rearrange("p (s e) -> p s e", e=HD),
                    op=AO.mult,
                )
                sraw = p_wk.tile([P, sw * H], BF16, tag="sraw")
                with nc.allow_low_precision(
                    reason="scores clipped to +-20; bf16 keeps ~0.3% rel"
                ):
                    nc.vector.tensor_reduce(
                        out=sraw[:],
                        in_=kq[:].rearrange("p (sh d) -> p sh d", d=D),
                        axis=mybir.AxisListType.X,
                        op=AO.add,
                    )
                nc.vector.tensor_scalar_max(out=sraw[:], in0=sraw[:], scalar1=-20.0)
                nc.vector.tensor_scalar_min(out=sraw[:], in0=sraw[:], scalar1=20.0)
                mS = p_wk.tile([P, sw * (HD + H)], BF16, tag="mS")
                mS3 = mS[:].rearrange("p (s f) -> p s f", f=HD + H)
                nc.scalar.activation(
                    out=mS3[:, :, HD : HD + H],
                    in_=sraw[:].rearrange("p (s h) -> p s h", h=H),
                    func=AF.Exp,
                    scale=0.25,
                )
                nc.vector.tensor_tensor(
                    out=mS3[:, :, 0:HD].rearrange("p s (h d) -> p s h d", d=D),
                    in0=kv3[:, :, HD : 2 * HD].rearrange("p s (h d) -> p s h d", d=D),
                    in1=mS3[:, :, HD : HD + H].unsqueeze(3).to_broadcast(
                        [P, sw, H, D]
                    ),
                    op=AO.mult,
                )
                # segment-sum via PE
                ps2 = p_2ps.tile([P, HD + H], F32, tag="ps2")
                for s in range(sw):
                    nc.tensor.matmul(
                        out=ps2[:],
                        lhsT=s13[:, s, :],
                        rhs=mS3[:, s, :],
                        start=(s == 0),
                        stop=(s == sw - 1),
                    )
                # epilogue: out = wV / (z + eps)
                zr = p_epi.tile([P, H], F32, tag="zr")
                nc.vector.tensor_scalar_add(
                    out=zr[:], in0=ps2[:, HD : HD + H], scalar1=1e-6
                )
                nc.vector.reciprocal(out=zr[:], in_=zr[:])
                outsb = p_epi.tile([P, HD], F32, tag="outsb")
                nc.vector.tensor_tensor(
                    out=outsb[:].rearrange("p (h d) -> p h d", d=D),
                    in0=ps2[:, 0:HD].rearrange("p (h d) -> p h d", d=D),
                    in1=zr[:].unsqueeze(2).to_broadcast([P, H, D]),
                    op=AO.mult,
                )
                nc.sync.dma_start(out=out_d[w * P : (w + 1) * P, :], in_=outsb[:])

            p_2ps_cm.__exit__(None, None, None)
            p_qeps_cm.__exit__(None, None, None)
            p_s2ps_cm.__exit__(None, None, None)

    nc.compile()
    return nc


_CACHE: dict = {}


def _get_program(cfg: Cfg):
    if cfg not in _CACHE:
        _CACHE[cfg] = build_program(cfg)
    return _CACHE[cfg]


def run(h, Wq, bq, Wk, bk, Wv, bv, src, dst, trace=False, **run_kwargs):
    """Returns (output, BassKernelResults)."""
    from concourse.bass_utils import run_bass_kernel_spmd

    h = np.asarray(h)
    cfg, shared, per_core = preprocess(
        h, np.asarray(Wq), np.asarray(bq), np.asarray(Wk), np.asarray(bk),
        np.asarray(Wv), np.asarray(bv), np.asarray(src), np.asarray(dst),
    )
    nc = _get_program(cfg)
    in_maps = [dict(shared, **pc) for pc in per_core]
    res = run_bass_kernel_spmd(
        nc, in_maps, core_ids=list(range(cfg.ncores)), trace=trace, **run_kwargs
    )
    outs = [res.results[c]["out"] for c in range(cfg.ncores)]
    return np.concatenate(outs, axis=0)[: cfg.n].astype(np.float32), res


def kernel(h, Wq, bq, Wk, bk, Wv, bv, src, dst, **_):
    out, _res = run(h, Wq, bq, Wk, bk, Wv, bv, src, dst, trace=False)
    return out
